# revision 42
# baseline (speedup 1.0000x reference)
"""CrossAttentionBlock kernel for Trainium2 (8 NeuronCores, batch-parallel).

Per-core problem (one batch sample): x, ctx [4096, 128] fp32.
  xn   = LayerNorm(x) * gamma + beta            (eps=1e-3, channel axis)
  q/k/v = (xn|ctx|ctx) @ W{q,k,v} + b{q,k,v}
  attn = softmax(q k^T / sqrt(128))
  out  = xn + (attn @ v) @ Wp + bp

Design notes:
- bf16 attention core (q/k/v/exp), fp32 LayerNorm + residual; softmax skips
  max-subtraction (scores/sqrt(U) ~ N(0,1), exp stays in range).
- Scores computed transposed (key-tokens on partitions) so attn@v with
  lhsT=exp-chunk, rhs=[v | ones] lands the *natural* [token, U] output with
  the softmax denominator as column 129 -- row sums cost zero extra PE work.
- gamma/beta folded into Wq/bq host-side (Wq' = diag(gamma) Wq,
  bq' = beta Wq + bq) so the Q path consumes raw normalized x.
- All layout transposes ride the DMA xbar (bf16), not the PE.
- PSUM budget: 4 banks score double-buffer + 3 banks attn-v accumulators
  (3 chunks/bank; only the bank's first chunk uses start=True because start
  clears has_written for the whole bank) + 1 bank for everything else.
- ACT runs only the 4 LN sqrts then the 128 exps (its in-order queue would
  stall exps behind anything else); residual scaling runs on GPSIMD.
"""

import sys

for _p in ("/opt/trn_rl_repo",):
    if _p not in sys.path:
        sys.path.insert(0, _p)

from contextlib import ExitStack

import numpy as np

import concourse.bass as bass
import concourse.tile as tile
from concourse import mybir
from concourse.bacc import Bacc
from concourse.bass_utils import run_bass_kernel_spmd
from concourse.masks import make_identity

P = 128
C = 128
U = 128
NT = 4096            # tokens per sample (64*64)
TILES = NT // P      # 32
NBW = 1024           # attention n-block width
NB = NT // NBW       # 4
EPS = 1e-3
SCALE = 1.0 / float(np.sqrt(U))

f32 = mybir.dt.float32
bf16 = mybir.dt.bfloat16
AF = mybir.ActivationFunctionType
OP = mybir.AluOpType


def build_bass():
    nc = Bacc(None)

    # host-side preprocessed parameters (see _prep_in_maps):
    #   cxb: context cast bf16; wq2 = diag(gamma) Wq bf16; bq2 = beta Wq + bq
    #   wv_aug = [Wv | 0] bf16; bvaug = [bv | 1]; betabp = beta + bp
    x_d = nc.declare_dram_parameter("x", [NT, C], f32, isOutput=False)
    cxb_d = nc.declare_dram_parameter("cxb", [NT, C], bf16, isOutput=False)
    gamma_d = nc.declare_dram_parameter("gamma", [C], f32, isOutput=False)
    betabp_d = nc.declare_dram_parameter("betabp", [C], f32, isOutput=False)
    wq_d = nc.declare_dram_parameter("wq2", [C, U], bf16, isOutput=False)
    bq_d = nc.declare_dram_parameter("bq2", [U], f32, isOutput=False)
    wk_d = nc.declare_dram_parameter("wk", [C, U], bf16, isOutput=False)
    bk_d = nc.declare_dram_parameter("bk", [U], f32, isOutput=False)
    wvaug_d = nc.declare_dram_parameter("wv_aug", [C, U + 1], bf16, isOutput=False)
    bvaug_d = nc.declare_dram_parameter("bvaug", [U + 1], f32, isOutput=False)
    wp_d = nc.declare_dram_parameter("wp", [U, U], bf16, isOutput=False)
    out_d = nc.declare_dram_parameter("out", [NT, C], f32, isOutput=True)

    def col(ap_1d):
        return ap_1d[:].rearrange("(p o) -> p o", o=1)

    def bcast(ap_1d):
        a = ap_1d[:]
        return bass.AP(tensor=a.tensor, offset=a.offset, ap=[[0, P], a.ap[0]])

    with ExitStack() as ctx:
        tc = ctx.enter_context(tile.TileContext(nc))

        consts = ctx.enter_context(tc.tile_pool(name="consts", bufs=1))
        big = ctx.enter_context(tc.tile_pool(name="big", bufs=1))
        tmp = ctx.enter_context(tc.tile_pool(name="tmp", bufs=4))
        epool = ctx.enter_context(tc.tile_pool(name="epool", bufs=3))
        rpool = ctx.enter_context(tc.tile_pool(name="rpool", bufs=3))
        scp = ctx.enter_context(tc.tile_pool(name="scp", bufs=2, space="PSUM"))
        avp = ctx.enter_context(tc.tile_pool(name="avp", bufs=3, space="PSUM"))
        prp = ctx.enter_context(tc.tile_pool(name="prp", bufs=1, space="PSUM"))

        # ---------------- constants ----------------
        wq_sb = consts.tile([C, U], bf16)
        nc.sync.dma_start(out=wq_sb, in_=wq_d[:, :])
        wk_sb = consts.tile([C, U], bf16)
        nc.sync.dma_start(out=wk_sb, in_=wk_d[:, :])
        wv_aug = consts.tile([C, U + 1], bf16)
        nc.sync.dma_start(out=wv_aug, in_=wvaug_d[:, :])
        wp_bf = consts.tile([U, U], bf16)
        nc.sync.dma_start(out=wp_bf, in_=wp_d[:, :])
        bq_col = consts.tile([P, 1], f32)
        nc.sync.dma_start(out=bq_col, in_=col(bq_d))
        bk_col = consts.tile([P, 1], f32)
        nc.sync.dma_start(out=bk_col, in_=col(bk_d))
        gamma_b = consts.tile([P, C], f32)
        nc.gpsimd.dma_start(out=gamma_b, in_=bcast(gamma_d))
        betabp_b = consts.tile([P, C], f32)
        nc.gpsimd.dma_start(out=betabp_b, in_=bcast(betabp_d))
        bvaug_b = consts.tile([P, U + 1], f32)
        nc.gpsimd.dma_start(out=bvaug_b, in_=bcast(bvaug_d))
        ident_b = consts.tile([P, P], bf16)
        make_identity(nc, ident_b)
        eps_t = consts.tile([P, 1], f32)
        nc.vector.memset(eps_t, EPS)

        # ---------------- persistent activations ----------------
        xall = big.tile([P, TILES, C], f32)    # x tiles, normalized in place
        xn_nat = big.tile([P, TILES, C], f32)  # gamma*xn + (beta+bp), natural
        xnT = big.tile([C, NT], bf16)          # raw normalized x, transposed
        cxT = big.tile([C, NT], bf16)
        qT = big.tile([U, NT], bf16)
        kT = big.tile([U, NT], bf16)
        vpr = big.tile([P, TILES, U + 1], bf16)  # v' = [v | 1] per key tile
        outT = big.tile([U, NT], bf16)           # normalized attn output^T

        # x loads first -- they gate the LN stats chain that gates everything
        GG = 8
        NG = TILES // GG
        for g in range(NG):
            nc.sync.dma_start(
                out=xall[:, g * GG:(g + 1) * GG, :],
                in_=x_d[g * GG * P:(g + 1) * GG * P, :].rearrange(
                    "(t p) c -> p t c", p=P))
        # context transposed via DMA xbar straight from DRAM; chunk 0 now
        # (gates kT[0] -> first scores), the rest after the first 8 chains
        def cx_transpose(cc):
            csl = slice(cc * 1024, (cc + 1) * 1024)
            nc.sync.dma_start_transpose(out=cxT[:, csl], in_=cxb_d[csl, :])

        cx_transpose(0)

        # ---------------- LN stats ----------------
        # ACT's in-order queue holds the sqrts ahead of the exps, so every
        # sqrt's bn dependency must resolve before the first exp fires.
        mvgs = [tmp.tile([P, GG, 2], f32, tag="mvg", name=f"mvg{g}")
                for g in range(NG)]
        rstgs = []

        def emit_bn(i):
            g, t = divmod(i, GG)
            st = tmp.tile([P, 6], f32, tag="st")
            nc.vector.bn_stats(out=st, in_=xall[:, i, :])
            nc.vector.bn_aggr(out=mvgs[g][:, t, :], in_=st)

        def emit_sqrt(g):
            rstg = tmp.tile([P, GG, 1], f32, tag="rstg", name=f"rstg{g}")
            nc.scalar.activation(out=rstg, in_=mvgs[g][:, :, 1:2],
                                 func=AF.Sqrt, bias=eps_t, scale=1.0)
            nc.vector.reciprocal(out=rstg, in_=rstg)
            rstgs.append(rstg)

        for t in range(GG):
            emit_bn(t)
        emit_sqrt(0)

        # ---------------- per-tile chains ----------------
        def tile_chain(i):
            g, t = divmod(i, GG)
            isl = slice(i * P, (i + 1) * P)
            nc.vector.tensor_scalar(
                out=xall[:, i, :], in0=xall[:, i, :],
                scalar1=mvgs[g][:, t, 0:1], scalar2=rstgs[g][:, t, :],
                op0=OP.subtract, op1=OP.mult)
            # bf16 copy then DMA-xbar transpose into xnT; emitted before the
            # residual ops -- it feeds the qT critical path.  All transposes
            # stay on the single sync HWDGE queue: a second queue races the
            # shared SDMA engines' xbar mode and corrupts data (observed).
            xnbf = tmp.tile([P, C], bf16, tag="xnbf")
            nc.vector.tensor_copy(out=xnbf, in_=xall[:, i, :])
            nc.sync.dma_start_transpose(out=xnT[:, isl], in_=xnbf)
            # residual (gamma*xn + beta+bp) on the otherwise-idle gpsimd
            nc.gpsimd.tensor_mul(out=xn_nat[:, i, :], in0=xall[:, i, :],
                                 in1=gamma_b)
            nc.gpsimd.tensor_add(out=xn_nat[:, i, :], in0=xn_nat[:, i, :],
                                 in1=betabp_b)
            # v' = [ctx@Wv + bv | 1]
            vp = prp.tile([P, U + 1], f32, tag="pro")
            nc.tensor.matmul(vp, lhsT=cxT[:, isl], rhs=wv_aug)
            nc.vector.tensor_tensor(out=vpr[:, i, :], in0=vp, in1=bvaug_b,
                                    op=OP.add)
            if i % 4 == 3:
                h = i // 4
                sl = slice(h * 512, (h + 1) * 512)
                kp = prp.tile([P, 512], f32, tag="pro")
                nc.tensor.matmul(kp, lhsT=wk_sb, rhs=cxT[:, sl])
                nc.vector.tensor_scalar_add(out=kT[:, sl], in0=kp,
                                            scalar1=bk_col)
                qp = prp.tile([P, 512], f32, tag="pro")
                nc.tensor.matmul(qp, lhsT=wq_sb, rhs=xnT[:, sl])
                nc.vector.tensor_scalar_add(out=qT[:, sl], in0=qp,
                                            scalar1=bq_col)

        # tiles 0-7 now (block 0 needs qT chunks 0-1), with groups 1-3's bn
        # stats interleaved (3 per chain) so the last sqrt resolves early;
        # tiles 8-31 are fed one-per-m-iteration into attention block 0
        # (24 iterations spare), keeping them off the in-order PE queue's
        # critical path.
        for i in range(GG):
            tile_chain(i)
        for cc in range(1, 4):
            cx_transpose(cc)
        for j in range(GG, TILES):
            emit_bn(j)
            if j % GG == GG - 1:
                emit_sqrt(j // GG)
        next_tile = [GG]

        def maybe_tile_chain():
            if next_tile[0] < TILES:
                tile_chain(next_tile[0])
                next_tile[0] += 1

        # ---------------- attention ----------------
        def av_matmuls(av, ex, m):
            for j in range(NBW // P):
                t, jj = divmod(j, 3)
                # start=True clears has_written for the WHOLE bank, so only
                # the bank's first chunk may use it; sibling chunks overwrite
                # at m=0 via the already-cleared bits.
                nc.tensor.matmul(
                    av[t][:, jj, :],
                    lhsT=ex[:, j * P:(j + 1) * P],
                    rhs=vpr[:, m, :],
                    start=(m == 0 and jj == 0), stop=(m == TILES - 1),
                    skip_group_check=True)

        def epilogue_chunk(avsb, j, c, tail=False):
            # normalize, transpose, project, residual, store for one
            # 128-token chunk; interleaved one-per-m-iteration into the NEXT
            # block so the in-order PE/DVE queues never see a burst.  The
            # final drain (tail=True) borrows the then-idle score PSUM slots
            # so its transpose/proj pairs double-buffer.
            tag = "sc" if tail else "pro"
            pool = scp if tail else prp
            rs = tmp.tile([P, 1], f32, tag="rs")
            nc.vector.reciprocal(out=rs, in_=avsb[:, j, U:U + 1])
            onat = tmp.tile([P, U], bf16, tag="onat")
            nc.vector.tensor_scalar_mul(out=onat, in0=avsb[:, j, 0:U],
                                        scalar1=rs)
            tp = pool.tile([P, P], bf16, tag=tag, name=f"tp{c}")
            nc.tensor.transpose(tp, onat, ident_b)
            nc.vector.tensor_copy(out=outT[:, c * P:(c + 1) * P], in_=tp)
            pj = pool.tile([P, U], f32, tag=tag, name=f"pj{c}")
            nc.tensor.matmul(pj, lhsT=outT[:, c * P:(c + 1) * P], rhs=wp_bf)
            res = rpool.tile([P, C], f32, tag="res")
            nc.vector.tensor_tensor(out=res, in0=pj, in1=xn_nat[:, c, :],
                                    op=OP.add)
            nc.sync.dma_start(out=out_d[c * P:(c + 1) * P, :], in_=res)

        pending = []
        for nb in range(NB):
            n0 = nb * NBW
            av = [avp.tile([P, 3, U + 1], f32, tag="av", name=f"av{nb}_{t}")
                  for t in range(3)]
            ex_prev = None
            for m in range(TILES):
                # software pipeline: issue scores(m) on PE before av(m-1) so
                # the in-order PE never stalls on exp(m-1)'s ACT op.
                sc = scp.tile([P, NBW], f32, tag="sc")
                with tc.high_priority():
                    for h in range(NBW // 512):
                        nc.tensor.matmul(
                            sc[:, h * 512:(h + 1) * 512],
                            lhsT=kT[:, m * P:(m + 1) * P],
                            rhs=qT[:, n0 + h * 512:n0 + (h + 1) * 512])
                if ex_prev is not None:
                    av_matmuls(av, ex_prev, m - 1)
                maybe_tile_chain()
                if pending:
                    pending.pop(0)()
                ex = epool.tile([P, NBW], bf16, tag="ex")
                nc.scalar.activation(out=ex, in_=sc, func=AF.Exp, scale=SCALE)
                ex_prev = ex
            av_matmuls(av, ex_prev, TILES - 1)
            # drain av PSUM to SBUF quickly so the next block's accumulators
            # aren't starved of banks; the rest of the epilogue is deferred
            avsb = rpool.tile([P, NBW // P, U + 1], f32, tag="avsb", bufs=2,
                              name=f"avsb{nb}")
            for j in range(NBW // P):
                t, jj = divmod(j, 3)
                nc.vector.tensor_copy(out=avsb[:, j, :], in_=av[t][:, jj, :])
            for j in range(NBW // P):
                c = nb * (NBW // P) + j
                pending.append(
                    lambda avsb=avsb, j=j, c=c, **kw:
                        epilogue_chunk(avsb, j, c, **kw))
        for f in pending:
            f(tail=True)

    nc.compile()
    return nc


_NC = None


def _get_nc():
    global _NC
    if _NC is None:
        _NC = build_bass()
    return _NC


def _prep_in_maps(inputs):
    import ml_dtypes

    B = 8
    f = lambda k: np.ascontiguousarray(np.asarray(inputs[k], np.float32))
    x = f("inputs").reshape(B, NT, C)
    cx = f("context").reshape(B, NT, C)
    gamma, beta, bp, bq = f("gamma"), f("beta"), f("bp"), f("bq")
    Wq = f("Wq")
    wv_aug = np.zeros((C, U + 1), np.float32)
    wv_aug[:, :U] = f("Wv")
    bvaug = np.concatenate([f("bv"), [1.0]]).astype(np.float32)
    shared = {
        "gamma": gamma,
        "betabp": (beta + bp).astype(np.float32),
        "wq2": (gamma[:, None] * Wq).astype(ml_dtypes.bfloat16),
        "bq2": (beta @ Wq + bq).astype(np.float32),
        "wk": f("Wk").astype(ml_dtypes.bfloat16),
        "bk": f("bk"),
        "wv_aug": wv_aug.astype(ml_dtypes.bfloat16),
        "bvaug": bvaug,
        "wp": f("Wp").astype(ml_dtypes.bfloat16),
    }
    return [
        {"x": np.ascontiguousarray(x[b]),
         "cxb": np.ascontiguousarray(cx[b]).astype(ml_dtypes.bfloat16),
         **shared}
        for b in range(B)
    ]


def kernel(**inputs):
    nc = _get_nc()
    in_maps = _prep_in_maps(inputs)
    res = run_bass_kernel_spmd(nc, in_maps, list(range(len(in_maps))))
    out = np.stack([res.results[b]["out"] for b in range(len(in_maps))])
    return out.reshape(len(in_maps), 64, 64, U).astype(np.float32)


if __name__ == "__main__":
    nc = build_bass()
    print("built ok")


# revision 43
# speedup vs baseline: 1.0041x; 1.0041x over previous
"""CrossAttentionBlock kernel for Trainium2 (8 NeuronCores, batch-parallel).

Per-core problem (one batch sample): x, ctx [4096, 128] fp32.
  xn   = LayerNorm(x) * gamma + beta            (eps=1e-3, channel axis)
  q/k/v = (xn|ctx|ctx) @ W{q,k,v} + b{q,k,v}
  attn = softmax(q k^T / sqrt(128))
  out  = xn + (attn @ v) @ Wp + bp

Design notes:
- bf16 attention core (q/k/v/exp), fp32 LayerNorm + residual; softmax skips
  max-subtraction (scores/sqrt(U) ~ N(0,1), exp stays in range).
- Scores computed transposed (key-tokens on partitions) so attn@v with
  lhsT=exp-chunk, rhs=[v | ones] lands the *natural* [token, U] output with
  the softmax denominator as column 129 -- row sums cost zero extra PE work.
- gamma/beta folded into Wq/bq host-side (Wq' = diag(gamma) Wq,
  bq' = beta Wq + bq) so the Q path consumes raw normalized x.
- All layout transposes ride the DMA xbar (bf16), not the PE.
- PSUM budget: 4 banks score double-buffer + 3 banks attn-v accumulators
  (3 chunks/bank; only the bank's first chunk uses start=True because start
  clears has_written for the whole bank) + 1 bank for everything else.
- ACT runs only the 4 LN sqrts then the 128 exps (its in-order queue would
  stall exps behind anything else); residual scaling runs on GPSIMD.
"""

import sys

for _p in ("/opt/trn_rl_repo",):
    if _p not in sys.path:
        sys.path.insert(0, _p)

from contextlib import ExitStack

import numpy as np

import concourse.bass as bass
import concourse.tile as tile
from concourse import mybir
from concourse.bacc import Bacc
from concourse.bass_utils import run_bass_kernel_spmd
from concourse.masks import make_identity

P = 128
C = 128
U = 128
NT = 4096            # tokens per sample (64*64)
TILES = NT // P      # 32
NBW = 1024           # attention n-block width
NB = NT // NBW       # 4
EPS = 1e-3
SCALE = 1.0 / float(np.sqrt(U))

f32 = mybir.dt.float32
bf16 = mybir.dt.bfloat16
AF = mybir.ActivationFunctionType
OP = mybir.AluOpType


def build_bass():
    nc = Bacc(None)

    # host-side preprocessed parameters (see _prep_in_maps):
    #   cxb: context cast bf16; wq2 = diag(gamma) Wq bf16; bq2 = beta Wq + bq
    #   wv_aug = [Wv | 0] bf16; bvaug = [bv | 1]; betabp = beta + bp
    x_d = nc.declare_dram_parameter("x", [NT, C], f32, isOutput=False)
    cxb_d = nc.declare_dram_parameter("cxb", [NT, C], bf16, isOutput=False)
    gamma_d = nc.declare_dram_parameter("gamma", [C], f32, isOutput=False)
    betabp_d = nc.declare_dram_parameter("betabp", [C], f32, isOutput=False)
    wq_d = nc.declare_dram_parameter("wq2", [C, U], bf16, isOutput=False)
    bq_d = nc.declare_dram_parameter("bq2", [U], f32, isOutput=False)
    wk_d = nc.declare_dram_parameter("wk", [C, U], bf16, isOutput=False)
    bk_d = nc.declare_dram_parameter("bk", [U], f32, isOutput=False)
    wvaug_d = nc.declare_dram_parameter("wv_aug", [C, U + 1], bf16, isOutput=False)
    bvaug_d = nc.declare_dram_parameter("bvaug", [U + 1], f32, isOutput=False)
    wp_d = nc.declare_dram_parameter("wp", [U, U], bf16, isOutput=False)
    out_d = nc.declare_dram_parameter("out", [NT, C], f32, isOutput=True)

    def col(ap_1d):
        return ap_1d[:].rearrange("(p o) -> p o", o=1)

    def bcast(ap_1d):
        a = ap_1d[:]
        return bass.AP(tensor=a.tensor, offset=a.offset, ap=[[0, P], a.ap[0]])

    with ExitStack() as ctx:
        tc = ctx.enter_context(tile.TileContext(nc))

        consts = ctx.enter_context(tc.tile_pool(name="consts", bufs=1))
        big = ctx.enter_context(tc.tile_pool(name="big", bufs=1))
        tmp = ctx.enter_context(tc.tile_pool(name="tmp", bufs=6))
        epool = ctx.enter_context(tc.tile_pool(name="epool", bufs=4))
        rpool = ctx.enter_context(tc.tile_pool(name="rpool", bufs=4))
        scp = ctx.enter_context(tc.tile_pool(name="scp", bufs=2, space="PSUM"))
        avp = ctx.enter_context(tc.tile_pool(name="avp", bufs=3, space="PSUM"))
        prp = ctx.enter_context(tc.tile_pool(name="prp", bufs=1, space="PSUM"))

        # ---------------- constants ----------------
        wq_sb = consts.tile([C, U], bf16)
        nc.sync.dma_start(out=wq_sb, in_=wq_d[:, :])
        wk_sb = consts.tile([C, U], bf16)
        nc.sync.dma_start(out=wk_sb, in_=wk_d[:, :])
        wv_aug = consts.tile([C, U + 1], bf16)
        nc.sync.dma_start(out=wv_aug, in_=wvaug_d[:, :])
        wp_bf = consts.tile([U, U], bf16)
        nc.sync.dma_start(out=wp_bf, in_=wp_d[:, :])
        bq_col = consts.tile([P, 1], f32)
        nc.sync.dma_start(out=bq_col, in_=col(bq_d))
        bk_col = consts.tile([P, 1], f32)
        nc.sync.dma_start(out=bk_col, in_=col(bk_d))
        gamma_b = consts.tile([P, C], f32)
        nc.gpsimd.dma_start(out=gamma_b, in_=bcast(gamma_d))
        betabp_b = consts.tile([P, C], f32)
        nc.gpsimd.dma_start(out=betabp_b, in_=bcast(betabp_d))
        bvaug_b = consts.tile([P, U + 1], f32)
        nc.gpsimd.dma_start(out=bvaug_b, in_=bcast(bvaug_d))
        ident_b = consts.tile([P, P], bf16)
        make_identity(nc, ident_b)
        eps_t = consts.tile([P, 1], f32)
        nc.vector.memset(eps_t, EPS)

        # ---------------- persistent activations ----------------
        xall = big.tile([P, TILES, C], f32)    # x tiles, normalized in place
        xn_nat = big.tile([P, TILES, C], f32)  # gamma*xn + (beta+bp), natural
        xnT = big.tile([C, NT], bf16)          # raw normalized x, transposed
        cxT = big.tile([C, NT], bf16)
        qT = big.tile([U, NT], bf16)
        kT = big.tile([U, NT], bf16)
        vpr = big.tile([P, TILES, U + 1], bf16)  # v' = [v | 1] per key tile
        outT = big.tile([U, NT], bf16)           # normalized attn output^T

        # x loads first -- they gate the LN stats chain that gates everything
        GG = 8
        NG = TILES // GG
        for g in range(NG):
            nc.sync.dma_start(
                out=xall[:, g * GG:(g + 1) * GG, :],
                in_=x_d[g * GG * P:(g + 1) * GG * P, :].rearrange(
                    "(t p) c -> p t c", p=P))
        # context transposed via DMA xbar straight from DRAM; chunk 0 now
        # (gates kT[0] -> first scores), the rest after the first 8 chains
        def cx_transpose(cc):
            csl = slice(cc * 1024, (cc + 1) * 1024)
            nc.sync.dma_start_transpose(out=cxT[:, csl], in_=cxb_d[csl, :])

        cx_transpose(0)

        # ---------------- LN stats ----------------
        # ACT's in-order queue holds the sqrts ahead of the exps, so every
        # sqrt's bn dependency must resolve before the first exp fires.
        mvgs = [tmp.tile([P, GG, 2], f32, tag="mvg", name=f"mvg{g}")
                for g in range(NG)]
        rstgs = []

        def emit_bn(i):
            g, t = divmod(i, GG)
            st = tmp.tile([P, 6], f32, tag="st")
            nc.vector.bn_stats(out=st, in_=xall[:, i, :])
            nc.vector.bn_aggr(out=mvgs[g][:, t, :], in_=st)

        def emit_sqrt(g):
            rstg = tmp.tile([P, GG, 1], f32, tag="rstg", name=f"rstg{g}")
            nc.scalar.activation(out=rstg, in_=mvgs[g][:, :, 1:2],
                                 func=AF.Sqrt, bias=eps_t, scale=1.0)
            nc.vector.reciprocal(out=rstg, in_=rstg)
            rstgs.append(rstg)

        for t in range(GG):
            emit_bn(t)
        emit_sqrt(0)

        # ---------------- per-tile chains ----------------
        def tile_chain(i):
            g, t = divmod(i, GG)
            isl = slice(i * P, (i + 1) * P)
            nc.vector.tensor_scalar(
                out=xall[:, i, :], in0=xall[:, i, :],
                scalar1=mvgs[g][:, t, 0:1], scalar2=rstgs[g][:, t, :],
                op0=OP.subtract, op1=OP.mult)
            # bf16 copy then DMA-xbar transpose into xnT; emitted before the
            # residual ops -- it feeds the qT critical path.  All transposes
            # stay on the single sync HWDGE queue: a second queue races the
            # shared SDMA engines' xbar mode and corrupts data (observed).
            xnbf = tmp.tile([P, C], bf16, tag="xnbf")
            nc.vector.tensor_copy(out=xnbf, in_=xall[:, i, :])
            nc.sync.dma_start_transpose(out=xnT[:, isl], in_=xnbf)
            # residual (gamma*xn + beta+bp) on the otherwise-idle gpsimd
            nc.gpsimd.tensor_mul(out=xn_nat[:, i, :], in0=xall[:, i, :],
                                 in1=gamma_b)
            nc.gpsimd.tensor_add(out=xn_nat[:, i, :], in0=xn_nat[:, i, :],
                                 in1=betabp_b)
            # v' = [ctx@Wv + bv | 1]
            vp = prp.tile([P, U + 1], f32, tag="pro")
            nc.tensor.matmul(vp, lhsT=cxT[:, isl], rhs=wv_aug)
            nc.vector.tensor_tensor(out=vpr[:, i, :], in0=vp, in1=bvaug_b,
                                    op=OP.add)
            if i % 4 == 3:
                h = i // 4
                sl = slice(h * 512, (h + 1) * 512)
                kp = prp.tile([P, 512], f32, tag="pro")
                nc.tensor.matmul(kp, lhsT=wk_sb, rhs=cxT[:, sl])
                nc.vector.tensor_scalar_add(out=kT[:, sl], in0=kp,
                                            scalar1=bk_col)
                qp = prp.tile([P, 512], f32, tag="pro")
                nc.tensor.matmul(qp, lhsT=wq_sb, rhs=xnT[:, sl])
                nc.vector.tensor_scalar_add(out=qT[:, sl], in0=qp,
                                            scalar1=bq_col)

        # tiles 0-7 now (block 0 needs qT chunks 0-1), with groups 1-3's bn
        # stats interleaved (3 per chain) so the last sqrt resolves early;
        # tiles 8-31 are fed one-per-m-iteration into attention block 0
        # (24 iterations spare), keeping them off the in-order PE queue's
        # critical path.
        for i in range(GG):
            tile_chain(i)
        for cc in range(1, 4):
            cx_transpose(cc)
        for j in range(GG, TILES):
            emit_bn(j)
            if j % GG == GG - 1:
                emit_sqrt(j // GG)
        next_tile = [GG]

        def maybe_tile_chain():
            if next_tile[0] < TILES:
                tile_chain(next_tile[0])
                next_tile[0] += 1

        # ---------------- attention ----------------
        def av_matmuls(av, ex, m):
            for j in range(NBW // P):
                t, jj = divmod(j, 3)
                # start=True clears has_written for the WHOLE bank, so only
                # the bank's first chunk may use it; sibling chunks overwrite
                # at m=0 via the already-cleared bits.
                nc.tensor.matmul(
                    av[t][:, jj, :],
                    lhsT=ex[:, j * P:(j + 1) * P],
                    rhs=vpr[:, m, :],
                    start=(m == 0 and jj == 0), stop=(m == TILES - 1),
                    skip_group_check=True)

        def epilogue_chunk(avsb, j, c, tail=False):
            # normalize, transpose, project, residual, store for one
            # 128-token chunk; interleaved one-per-m-iteration into the NEXT
            # block so the in-order PE/DVE queues never see a burst.  The
            # final drain (tail=True) borrows the then-idle score PSUM slots
            # so its transpose/proj pairs double-buffer.
            tag = "sc" if tail else "pro"
            pool = scp if tail else prp
            rs = tmp.tile([P, 1], f32, tag="rs")
            nc.vector.reciprocal(out=rs, in_=avsb[:, j, U:U + 1])
            onat = tmp.tile([P, U], bf16, tag="onat")
            nc.vector.tensor_scalar_mul(out=onat, in0=avsb[:, j, 0:U],
                                        scalar1=rs)
            tp = pool.tile([P, P], bf16, tag=tag, name=f"tp{c}")
            nc.tensor.transpose(tp, onat, ident_b)
            nc.vector.tensor_copy(out=outT[:, c * P:(c + 1) * P], in_=tp)
            pj = pool.tile([P, U], f32, tag=tag, name=f"pj{c}")
            nc.tensor.matmul(pj, lhsT=outT[:, c * P:(c + 1) * P], rhs=wp_bf)
            res = rpool.tile([P, C], f32, tag="res")
            nc.vector.tensor_tensor(out=res, in0=pj, in1=xn_nat[:, c, :],
                                    op=OP.add)
            nc.sync.dma_start(out=out_d[c * P:(c + 1) * P, :], in_=res)

        pending = []
        for nb in range(NB):
            n0 = nb * NBW
            av = [avp.tile([P, 3, U + 1], f32, tag="av", name=f"av{nb}_{t}")
                  for t in range(3)]
            ex_prev = None
            for m in range(TILES):
                # software pipeline: issue scores(m) on PE before av(m-1) so
                # the in-order PE never stalls on exp(m-1)'s ACT op.
                sc = scp.tile([P, NBW], f32, tag="sc")
                with tc.high_priority():
                    for h in range(NBW // 512):
                        nc.tensor.matmul(
                            sc[:, h * 512:(h + 1) * 512],
                            lhsT=kT[:, m * P:(m + 1) * P],
                            rhs=qT[:, n0 + h * 512:n0 + (h + 1) * 512])
                if ex_prev is not None:
                    av_matmuls(av, ex_prev, m - 1)
                maybe_tile_chain()
                if pending:
                    pending.pop(0)()
                ex = epool.tile([P, NBW], bf16, tag="ex")
                nc.scalar.activation(out=ex, in_=sc, func=AF.Exp, scale=SCALE)
                ex_prev = ex
            av_matmuls(av, ex_prev, TILES - 1)
            # drain av PSUM to SBUF quickly so the next block's accumulators
            # aren't starved of banks; the rest of the epilogue is deferred
            avsb = rpool.tile([P, NBW // P, U + 1], f32, tag="avsb", bufs=2,
                              name=f"avsb{nb}")
            for j in range(NBW // P):
                t, jj = divmod(j, 3)
                nc.vector.tensor_copy(out=avsb[:, j, :], in_=av[t][:, jj, :])
            for j in range(NBW // P):
                c = nb * (NBW // P) + j
                pending.append(
                    lambda avsb=avsb, j=j, c=c, **kw:
                        epilogue_chunk(avsb, j, c, **kw))
        for f in pending:
            f(tail=True)

    nc.compile()
    return nc


_NC = None


def _get_nc():
    global _NC
    if _NC is None:
        _NC = build_bass()
    return _NC


def _prep_in_maps(inputs):
    import ml_dtypes

    B = 8
    f = lambda k: np.ascontiguousarray(np.asarray(inputs[k], np.float32))
    x = f("inputs").reshape(B, NT, C)
    cx = f("context").reshape(B, NT, C)
    gamma, beta, bp, bq = f("gamma"), f("beta"), f("bp"), f("bq")
    Wq = f("Wq")
    wv_aug = np.zeros((C, U + 1), np.float32)
    wv_aug[:, :U] = f("Wv")
    bvaug = np.concatenate([f("bv"), [1.0]]).astype(np.float32)
    shared = {
        "gamma": gamma,
        "betabp": (beta + bp).astype(np.float32),
        "wq2": (gamma[:, None] * Wq).astype(ml_dtypes.bfloat16),
        "bq2": (beta @ Wq + bq).astype(np.float32),
        "wk": f("Wk").astype(ml_dtypes.bfloat16),
        "bk": f("bk"),
        "wv_aug": wv_aug.astype(ml_dtypes.bfloat16),
        "bvaug": bvaug,
        "wp": f("Wp").astype(ml_dtypes.bfloat16),
    }
    return [
        {"x": np.ascontiguousarray(x[b]),
         "cxb": np.ascontiguousarray(cx[b]).astype(ml_dtypes.bfloat16),
         **shared}
        for b in range(B)
    ]


def kernel(**inputs):
    nc = _get_nc()
    in_maps = _prep_in_maps(inputs)
    res = run_bass_kernel_spmd(nc, in_maps, list(range(len(in_maps))))
    out = np.stack([res.results[b]["out"] for b in range(len(in_maps))])
    return out.reshape(len(in_maps), 64, 64, U).astype(np.float32)


if __name__ == "__main__":
    nc = build_bass()
    print("built ok")
